# revision 24
# baseline (speedup 1.0000x reference)
"""Row-wise cosine similarity on 8 TRN2 NeuronCores.

out[n] = sum_d(p[n,d]*h[n,d]) / (max(||p[n]||,eps) * max(||h[n]||,eps))
with N=65536, D=1024, eps=1e-12 (torch F.normalize semantics).

Sharding: rows split evenly across 8 cores (data parallel, no comms).
Per core (8192 rows): rows are laid out as r = partition*64 + tile, so a
[128, G, 1024] SBUF tile loads G*4KB contiguous bytes per partition and
the per-row results land in a [128, 64] SBUF tile that DMAs out in one
contiguous-per-partition transfer (no on-chip transpose needed).

Raw bass (no Tile scheduler): the walrus codegen in this toolchain
accepts at most ONE sync wait per instruction, which Tile's automatic
semaphore assignment violates for this dataflow. Hand-placed counting
semaphores keep every instruction at <=1 wait by exploiting transitivity:
the DVE group-completion increment happens after a wait on the ACT
group-completion sem, so DMAs recycling a buffer slot only wait on the
DVE sem.

Engine balance (HW-measured: ACTIVATE+accum-read 1.41us, DVE
scalar_tensor_tensor+accum 1.29us per [128,1024] tile): ACT computes
||p||^2 (Square + row-accumulate), DVE computes p.h and ||h||^2 via
scalar_tensor_tensor accum_out, with the last few tiles' ||h||^2 on
ACT so DVE drains quickly after the final DMA. All input loads issue
from the SP HWDGE ring; group sizes taper at both ends. Epilogue: ph * rsqrt(pp*hh) with ACT sqrt + DVE reciprocal + one
Newton-Raphson step.
"""

import numpy as np

try:
    import concourse.bass as bass
except ImportError:  # fresh grading dir: toolchain lives in /opt
    import sys

    sys.path.insert(0, "/opt/trn_rl_repo")
    import concourse.bass as bass

from contextlib import ExitStack

from concourse import mybir
from concourse.bass_utils import run_bass_kernel_spmd

N, D = 65536, 1024
NCORES = 8
ROWS = N // NCORES  # 8192 rows per core
P = 128  # SBUF partitions
GMAX = 4  # row-tiles per full group: one [128, G*1024] f32 load = 2 MB
B = 6  # in-flight group buffers
EPS2 = 1e-24  # eps^2; max(||x||,eps) == sqrt(max(||x||^2, eps^2)) here

_NC_CACHE = {}


def _group_sizes(T, gmax):
    """Slightly smaller first group (earlier compute start without
    starving the DMA issue pipeline), full-size body, then a short taper
    so the post-stream compute drain is small."""
    front = [t for t in (2,) if t < gmax]
    back = [t for t in (2, 1, 1) if t < gmax]
    body = T - sum(front) - sum(back)
    sizes = [gmax] * (body // gmax)
    rem = body % gmax
    if rem:
        sizes.append(rem)
    sizes = front + sizes + back
    assert sum(sizes) == T
    return sizes


def _hh_on_act(t, T):
    """Which tiles' ||h||^2 runs on ACT instead of DVE. Keeping ACT off
    the h-half avoids SBUF read contention with DVE (measured: both
    engines on h inflates every op ~18%), so only the tail tiles move to
    ACT — that halves DVE's work for the final tiles and shrinks the
    compute drain after the last DMA byte lands."""
    return t >= T - 8


def _build_bass(rows=ROWS, gmax=GMAX, b=B, unique_junk=False, detect_races=False,
                taper=True):
    fp32 = mybir.dt.float32
    Sq = mybir.ActivationFunctionType.Square
    Sqrt = mybir.ActivationFunctionType.Sqrt
    mult = mybir.AluOpType.mult
    T = rows // P
    B = b
    sizes = _group_sizes(T, gmax) if taper else [gmax] * (T // gmax)
    starts = [sum(sizes[:i]) for i in range(len(sizes))]
    NG = len(sizes)

    nc = bass.Bass(detect_race_conditions=detect_races)
    prem = nc.declare_dram_parameter("premise", [rows, D], fp32, isOutput=False)
    hyp = nc.declare_dram_parameter("hypothesis", [rows, D], fp32, isOutput=False)
    outp = nc.declare_dram_parameter("out", [rows], fp32, isOutput=True)

    # row r = p*T + t: partition-strided input DMA, contiguous output DMA
    prem3 = prem[:].rearrange("(p t) d -> p t d", p=P)
    hyp3 = hyp[:].rearrange("(p t) d -> p t d", p=P)
    out2 = outp[:].rearrange("(p t) -> p t", p=P)

    # junk: mandatory full-size outputs of accumulate ops; values unused.
    # unique_junk gives every instruction its own slice (race-detector-clean
    # validation builds only — too big for the full problem size).
    na = 2 * T if unique_junk else 1
    nv = 2 * T if unique_junk else 1

    with ExitStack() as mem:
        xs = [
            mem.enter_context(nc.sbuf_tensor(f"xs{i}", [P, 2, gmax, D], fp32))
            for i in range(B)
        ]
        junk_a = mem.enter_context(nc.sbuf_tensor("junk_a", [P, na, D], fp32))
        junk_v = mem.enter_context(nc.sbuf_tensor("junk_v", [P, nv, D], fp32))
        r_pp = mem.enter_context(nc.sbuf_tensor("r_pp", [P, T], fp32))
        r_hh = mem.enter_context(nc.sbuf_tensor("r_hh", [P, T], fp32))
        r_ph = mem.enter_context(nc.sbuf_tensor("r_ph", [P, T], fp32))
        d2 = mem.enter_context(nc.sbuf_tensor("d2", [P, T], fp32))
        sd = mem.enter_context(nc.sbuf_tensor("sd", [P, T], fp32))
        yv = mem.enter_context(nc.sbuf_tensor("yv", [P, T], fp32))
        t1 = mem.enter_context(nc.sbuf_tensor("t1", [P, T], fp32))
        res = mem.enter_context(nc.sbuf_tensor("res", [P, T], fp32))

        with ExitStack() as semctx:
            s_dma_p = [
                semctx.enter_context(nc.semaphore(f"s_dma_p{i}")) for i in range(8)
            ]
            s_dma_h = [
                semctx.enter_context(nc.semaphore(f"s_dma_h{i}")) for i in range(8)
            ]
            s_act = semctx.enter_context(nc.semaphore("s_act"))
            s_dve = semctx.enter_context(nc.semaphore("s_dve"))
            s_ch = semctx.enter_context(nc.semaphore("s_ch"))
            s_ep2 = semctx.enter_context(nc.semaphore("s_ep2"))
            s_res = semctx.enter_context(nc.semaphore("s_res"))
            s_out = semctx.enter_context(nc.semaphore("s_out"))

            def pslice(g):
                s0, g0 = starts[g], sizes[g]
                return prem3[:, s0 : s0 + g0, :]

            def hslice(g):
                s0, g0 = starts[g], sizes[g]
                return hyp3[:, s0 : s0 + g0, :]

            with nc.Block() as block:

                @block.sync
                def _(eng: bass.BassEngine):
                    for g in range(NG):
                        if g >= B:
                            # DVE inc implies ACT done too (transitive)
                            eng.wait_ge(s_dve, g - B + 1)
                        eng.dma_start(
                            out=xs[g % B][:, 0, : sizes[g], :], in_=pslice(g)
                        ).then_inc(s_dma_p[g % 8], 16)
                        eng.dma_start(
                            out=xs[g % B][:, 1, : sizes[g], :], in_=hslice(g)
                        ).then_inc(s_dma_h[g % 8], 16)
                    eng.wait_ge(s_res, 1)
                    eng.dma_start(out=out2, in_=res[:]).then_inc(s_out, 16)
                    eng.wait_ge(s_out, 16)

                @block.scalar
                def _(eng: bass.BassEngine):
                    for g in range(NG):
                        eng.wait_ge(s_dma_p[g % 8], 16 * (g // 8 + 1))
                        sl = xs[g % B]
                        hh_mine = [
                            j for j in range(sizes[g]) if _hh_on_act(starts[g] + j, T)
                        ]
                        last = ("hh", hh_mine[-1]) if hh_mine else ("pp", sizes[g] - 1)
                        for j in range(sizes[g]):
                            t = starts[g] + j
                            ins = eng.activation(
                                out=junk_a[:, t % na, :],
                                in_=sl[:, 0, j, :],
                                func=Sq,
                                accum_out=r_pp[:, t : t + 1],
                            )
                            if last == ("pp", j):
                                ins.then_inc(s_act, 1)
                        if hh_mine:
                            eng.wait_ge(s_dma_h[g % 8], 16 * (g // 8 + 1))
                            for j in hh_mine:
                                t = starts[g] + j
                                ins = eng.activation(
                                    out=junk_a[:, (t + T) % na, :],
                                    in_=sl[:, 1, j, :],
                                    func=Sq,
                                    accum_out=r_hh[:, t : t + 1],
                                )
                                if last == ("hh", j):
                                    ins.then_inc(s_act, 1)
                    # epilogue: sqrt of pp*hh (after DVE built d2 = 2nd chain inc)
                    eng.wait_ge(s_ch, 2)
                    eng.activation(out=sd[:], in_=d2[:], func=Sqrt).then_inc(s_ep2, 1)

                @block.vector
                def _(eng: bass.BassEngine):
                    mx = mybir.AluOpType.max
                    add = mybir.AluOpType.add
                    for g in range(NG):
                        # p(g) and h(g) are issued back-to-back on the same
                        # SP HWDGE queue; each SDMA engine drains its ring
                        # FIFO, so 16 incs on the h sem imply p landed too.
                        eng.wait_ge(s_dma_h[g % 8], 16 * (g // 8 + 1))
                        sl = xs[g % B]
                        ops = []  # (kind, j, t)
                        for j in range(sizes[g]):
                            t = starts[g] + j
                            ops.append(("ph", j, t))
                            if not _hh_on_act(t, T):
                                ops.append(("hh", j, t))
                        for k, (kind, j, t) in enumerate(ops):
                            if k == len(ops) - 1:
                                # DVE completion of group g implies ACT done
                                eng.wait_ge(s_act, g + 1)
                            if kind == "ph":
                                ins = eng.scalar_tensor_tensor(
                                    out=junk_v[:, (2 * t) % nv, :],
                                    in0=sl[:, 0, j, :],
                                    scalar=1.0,
                                    in1=sl[:, 1, j, :],
                                    op0=mult,
                                    op1=mult,
                                    accum_out=r_ph[:, t : t + 1],
                                )
                            else:
                                ins = eng.scalar_tensor_tensor(
                                    out=junk_v[:, (2 * t + 1) % nv, :],
                                    in0=sl[:, 1, j, :],
                                    scalar=1.0,
                                    in1=sl[:, 1, j, :],
                                    op0=mult,
                                    op1=mult,
                                    accum_out=r_hh[:, t : t + 1],
                                )
                            if k == len(ops) - 1:
                                ins.then_inc(s_dve, 1)
                    # epilogue: res = ph * rsqrt(max(pp,e)*max(hh,e)).
                    # DVE pipelines same-engine dependent ops, so every
                    # same-engine RAW needs a sem; s_ch counts epilogue
                    # DVE completions. s_act >= NG was observed in the
                    # g-loop; s_dve >= NG orders the r_ph/r_hh reads
                    # behind the last stt writes.
                    eng.wait_ge(s_dve, NG)
                    eng.tensor_scalar_max(
                        out=r_hh[:], in0=r_hh[:], scalar1=EPS2
                    ).then_inc(s_ch, 1)
                    eng.wait_ge(s_ch, 1)
                    eng.scalar_tensor_tensor(
                        out=d2[:], in0=r_pp[:], scalar=EPS2, in1=r_hh[:],
                        op0=mx, op1=mult,
                    ).then_inc(s_ch, 1)
                    eng.wait_ge(s_ep2, 1)
                    eng.reciprocal(out=yv[:], in_=sd[:]).then_inc(s_ch, 1)
                    # Newton step for rsqrt: y *= 1.5 - 0.5*d2*y*y
                    eng.wait_ge(s_ch, 3)
                    eng.tensor_mul(t1[:], yv[:], yv[:]).then_inc(s_ch, 1)
                    eng.wait_ge(s_ch, 4)
                    eng.scalar_tensor_tensor(
                        out=t1[:], in0=d2[:], scalar=-0.5, in1=t1[:],
                        op0=mult, op1=mult,
                    ).then_inc(s_ch, 1)
                    eng.wait_ge(s_ch, 5)
                    eng.scalar_tensor_tensor(
                        out=yv[:], in0=t1[:], scalar=1.5, in1=yv[:],
                        op0=add, op1=mult,
                    ).then_inc(s_ch, 1)
                    eng.wait_ge(s_ch, 6)
                    eng.tensor_mul(res[:], r_ph[:], yv[:]).then_inc(s_res, 1)

    return nc


def _get_nc():
    if "nc" not in _NC_CACHE:
        _NC_CACHE["nc"] = _build_bass()
    return _NC_CACHE["nc"]


def _run(premise, hypothesis, trace=False, **kwargs):
    premise = np.ascontiguousarray(np.asarray(premise, dtype=np.float32))
    hypothesis = np.ascontiguousarray(np.asarray(hypothesis, dtype=np.float32))
    assert premise.shape == (N, D) and hypothesis.shape == (N, D)
    nc = _get_nc()
    in_maps = [
        {
            "premise": premise[c * ROWS : (c + 1) * ROWS],
            "hypothesis": hypothesis[c * ROWS : (c + 1) * ROWS],
        }
        for c in range(NCORES)
    ]
    r = run_bass_kernel_spmd(nc, in_maps, list(range(NCORES)), trace=trace, **kwargs)
    out = np.concatenate([r.results[c]["out"] for c in range(NCORES)])
    return out, r


def kernel(premise, hypothesis):
    out, _ = _run(premise, hypothesis)
    return out
